# revision 20
# baseline (speedup 1.0000x reference)
"""Trainium2 Bass kernel for nn_AutoregulatedContinuum.

Data-parallel over 8 NeuronCores: x sharded along batch N; V_slow/gate
params replicated.  W_fast is all zeros in this model family, so the
Hebbian branch contributes exactly zero and the computation reduces to

  v    = x @ V_w.T
  out  = sigmoid(v @ gate_w.T + gate_b) * strength * v

where strength = ctrl[0] of the regulator MLP driven by three global
scalars (var(x), mean|v|, ||W_slow||_F).

Single-phase device kernel (per core):
  - V_w.T resident in SBUF as bf16 (8 MB, eight 1 MB grouped DMAs on the
    sync HWDGE ring), x streamed as bf16 lhsT tiles on the gpsimd SWDGE
    ring (so prefetch triggers never queue behind scalar-engine compute);
    xi pool depth 3 plus FIFO ordering stagger the x/param transfers so
    only ~1.5 MB competes with the weight stream at the head
  - v row-tile accumulated in PSUM (fp32) via bf16 matmuls
  - per tile, straight out of PSUM: sum|v| (ACT accumulate), gate logit
    (DVE mul+reduce vs replicated gate_w), sigmoid (ACT), and
    pre = sigmoid(logit) * v  (DVE scalar-mul, fused PSUM->SBUF copy)
  - row-tiles 0 and 1 are interleaved over the contraction loop so the
    PE has ~2x work per arriving weight group while V_w.T streams in,
    then evacuated via fast ACT copies so PSUM frees immediately
  - pre spilled to HBM fp32; per-core sum|v| partials returned

Everything that needs the cross-core reduction (regulator MLP, the
global scalar `strength`) runs on the host during unsharding: stress and
fatigue come from the fp32 inputs directly, excitation from the summed
per-core partials, and the final output is strength * pre (a scalar
multiply applied while gathering shards).

If W_fast is ever nonzero we fall back to a host reference.
"""

import numpy as np

DIM = 2048
N = 16384
NCORES = 8
RPC = N // NCORES            # rows per core
ITILES = RPC // 128          # 16 row-tiles per core
KTILES = DIM // 128          # 16 contraction tiles
JCH = 4                      # output column chunks of 512
WGRP = 2                     # weight k-tiles per grouped DMA
LN_EPS = 1e-5

_CACHE = {}


def _build_program():
    import concourse.bacc as bacc
    import concourse.tile as tile
    import concourse.mybir as mybir

    F32 = mybir.dt.float32
    BF16 = mybir.dt.bfloat16
    AX = mybir.AxisListType
    ALU = mybir.AluOpType
    ACT = mybir.ActivationFunctionType

    nc = bacc.Bacc("TRN2", target_bir_lowering=False, debug=False,
                   num_devices=NCORES)

    # xtl[i*128 + p, t*128 + r] = x_shard[i*128 + r, t*128 + p]
    # i.e. row-block i holds the 16 k-tile lhsT operands for that row tile,
    # contiguous so each per-tile DMA is one 512 KB linear transfer.
    xtl = nc.dram_tensor("xtl", [RPC, DIM], BF16, kind="ExternalInput").ap()
    vwt = nc.dram_tensor("vwt", [DIM, DIM], BF16, kind="ExternalInput").ap()
    gw1 = nc.dram_tensor("gw1", [1, DIM], F32, kind="ExternalInput").ap()
    gbrep = nc.dram_tensor("gbrep", [128, 1], F32, kind="ExternalInput").ap()
    pre = nc.dram_tensor("pre", [RPC, DIM], F32, kind="ExternalOutput").ap()
    accav = nc.dram_tensor("accav", [128, ITILES + 1], F32,
                           kind="ExternalOutput").ap()

    vwt3 = vwt.rearrange("(t p) d -> p t d", p=128)      # [128, KTILES, DIM]

    with tile.TileContext(nc) as tc:
        with tc.tile_pool(name="const", bufs=1) as cst:
            gwr = cst.tile([128, DIM], F32)
            gbr = cst.tile([128, 1], F32)
            gw1s = cst.tile([1, DIM], F32)
            ones1 = cst.tile([1, 128], F32)
            acc_av = cst.tile([128, ITILES + 1], F32)
            nc.vector.memset(ones1[:], 1.0)
            # gate params are tiny (8 KB + 512 B): fetch on the scalar ring
            # and replicate gate_w across partitions on-device with a K=1
            # ones-matmul, instead of shipping a 1 MB replicated tensor
            # through the bandwidth-critical head window.
            nc.scalar.dma_start(gw1s[:], gw1[:, :])
            nc.scalar.dma_start(gbr[:], gbrep[:, :])

            with tc.tile_pool(name="wpool", bufs=1) as wp, \
                 tc.tile_pool(name="xtp", bufs=3) as xtp, \
                 tc.tile_pool(name="vcp", bufs=2) as vcp, \
                 tc.tile_pool(name="scra", bufs=2) as scra, \
                 tc.tile_pool(name="scrp", bufs=2) as scrp, \
                 tc.tile_pool(name="pop", bufs=3) as pop, \
                 tc.tile_pool(name="gp", bufs=4) as gp, \
                 tc.tile_pool(name="psv", bufs=2, space="PSUM") as psv:
                # resident weights: one 8 MB SBUF tile, grouped DMAs on the
                # sync ring.  The first two k-tiles ship individually so
                # the PE's first accumulation steps unblock sooner.
                w_all = wp.tile([128, KTILES * DIM], BF16, tag="w_all")
                w3 = w_all[:].rearrange("p (t d) -> p t d", t=KTILES)
                wgroups = [(t, t + 1) for t in range(KTILES)]
                for t0, t1 in wgroups:
                    nc.sync.dma_start(w3[:, t0:t1, :], vwt3[:, t0:t1, :])

                def rhs(t, j):
                    return w_all[:, t * DIM + j * 512: t * DIM + j * 512 + 512]

                # x row-tiles stream on the gpsimd SWDGE ring; triggers
                # there are never blocked behind ACT/DVE compute.  With
                # pool depth 3, xi3's trigger waits (at the queue head)
                # for tile 0's matmuls, which also delays gwr/gbr and
                # xi4+ -- keeping the head bandwidth for the weights.
                xi_t = []
                for i in range(ITILES):
                    xi = xtp.tile([128, DIM], BF16, tag="xi")
                    rows = slice(i * 128, (i + 1) * 128)
                    if i < 3:
                        # first tiles: ship the first 4 k-slices separately
                        # so the PE's t=0 matmuls unblock ~5us earlier
                        nc.gpsimd.dma_start(xi[:, 0:512], xtl[rows, 0:512])
                        nc.gpsimd.dma_start(xi[:, 512:], xtl[rows, 512:])
                    else:
                        nc.gpsimd.dma_start(xi[:], xtl[rows, :])
                    xi_t.append(xi)

                def post_chain(src, i, split=1, abs_last=False):
                    """sum|v|, gate logit, sigmoid, pre = sig*v, spill."""
                    def do_abs():
                        sa = scra.tile([128, DIM], F32, tag="sa")
                        nc.scalar.activation(sa[:], src[:], ACT.Abs,
                                             accum_out=acc_av[:, i:i + 1])
                    if not abs_last:
                        do_abs()
                    scr = scrp.tile([128, DIM], F32, tag="scr")
                    nc.vector.tensor_mul(scr[:], src[:], gwr[:])
                    gl = gp.tile([128, 1], F32, tag="gl")
                    nc.vector.tensor_reduce(gl[:], scr[:], axis=AX.X,
                                            op=ALU.add)
                    gs = gp.tile([128, 1], F32, tag="gs")
                    nc.scalar.activation(gs[:], gl[:], ACT.Sigmoid,
                                         bias=gbr[:, 0:1])
                    po = pop.tile([128, DIM], F32, tag="po")
                    rows = slice(i * 128, (i + 1) * 128)
                    w = DIM // split
                    for s in range(split):
                        sl = slice(s * w, (s + 1) * w)
                        nc.vector.tensor_scalar_mul(po[:, sl], src[:, sl],
                                                    gs[:, 0:1])
                        nc.sync.dma_start(pre[rows, sl], po[:, sl])
                    if abs_last:
                        # off the drain-critical path: overlaps the final
                        # output DMAs instead of delaying the gate chain
                        do_abs()

                # on-device broadcast of gate_w: [1,DIM] -> [128,DIM] via a
                # K=1 ones-matmul; runs in the otherwise-idle PE window at
                # the head and frees its PSUM slot within ~11 us
                bpv = psv.tile([128, DIM], F32, tag="pv")
                for j in range(JCH):
                    nc.tensor.matmul(bpv[:, j * 512:(j + 1) * 512],
                                     ones1[0:1, :],
                                     gw1s[0:1, j * 512:(j + 1) * 512],
                                     start=True, stop=True)
                nc.vector.tensor_copy(gwr[:], bpv[:])

                # ---- tiles 0,1: interleaved over t so the PE has two
                # tiles of matmul work per arriving weight group ----
                pv0 = psv.tile([128, DIM], F32, tag="pv")
                pv1 = psv.tile([128, DIM], F32, tag="pv")
                for t in range(KTILES):
                    for xi, pv in ((xi_t[0], pv0), (xi_t[1], pv1)):
                        lhsT = xi[:, t * 128:(t + 1) * 128]
                        for j in range(JCH):
                            nc.tensor.matmul(pv[:, j * 512:(j + 1) * 512],
                                             lhsT, rhs(t, j),
                                             start=(t == 0),
                                             stop=(t == KTILES - 1))
                # fast evacuation (ACT copies) so tiles 2,3 get PSUM back
                vc0 = vcp.tile([128, DIM], F32, tag="vc")
                nc.scalar.copy(vc0[:], pv0[:])
                vc1 = vcp.tile([128, DIM], F32, tag="vc")
                nc.scalar.copy(vc1[:], pv1[:])
                post_chain(vc0, 0)
                post_chain(vc1, 1)

                # ---- tiles 2..14: steady state, post straight from PSUM
                for i in range(2, ITILES - 1):
                    xi = xi_t[i]
                    pv = psv.tile([128, DIM], F32, tag="pv")
                    for t in range(KTILES):
                        lhsT = xi[:, t * 128:(t + 1) * 128]
                        for j in range(JCH):
                            nc.tensor.matmul(pv[:, j * 512:(j + 1) * 512],
                                             lhsT, rhs(t, j),
                                             start=(t == 0),
                                             stop=(t == KTILES - 1))
                    post_chain(pv, i, split=1)

                # ---- last tile: two column halves in SEPARATE psum tiles
                # (separate Tile objects, so half A's readers don't block
                # half B's matmuls via the whole-tile WAR guard) -- half
                # A's gate-dot runs under half B's matmul shadow, cutting
                # the post-matmul drain chain ----
                i = ITILES - 1
                xi = xi_t[i]
                rows = slice(i * 128, (i + 1) * 128)
                glh = gp.tile([128, 2], F32, tag="glh", bufs=1)
                halves = []
                for h in range(2):
                    pvh = psv.tile([128, DIM], F32, tag="pv")
                    for t in range(KTILES):
                        lhsT = xi[:, t * 128:(t + 1) * 128]
                        for j in range(2):
                            nc.tensor.matmul(pvh[:, j * 512:(j + 1) * 512],
                                             lhsT, rhs(t, 2 * h + j),
                                             start=(t == 0),
                                             stop=(t == KTILES - 1))
                    halves.append(pvh)
                    cols = slice(h * 1024, (h + 1) * 1024)
                    # gate-dot partial for this half (half 0 overlaps the
                    # half-1 matmul stream); |v| for half 1 is deferred
                    # below, off the drain-critical path
                    scr = scrp.tile([128, DIM], F32, tag="scr")
                    nc.vector.tensor_mul(scr[:, 0:1024], pvh[:, 0:1024],
                                         gwr[:, cols])
                    nc.vector.tensor_reduce(glh[:, h:h + 1], scr[:, 0:1024],
                                            axis=AX.X, op=ALU.add)
                    if h == 0:
                        sa = scra.tile([128, DIM], F32, tag="sa")
                        nc.scalar.activation(sa[:, 0:1024], pvh[:, 0:1024],
                                             ACT.Abs,
                                             accum_out=acc_av[:, i:i + 1])
                gl = gp.tile([128, 1], F32, tag="gl")
                nc.vector.tensor_reduce(gl[:], glh[:], axis=AX.X, op=ALU.add)
                gs = gp.tile([128, 1], F32, tag="gs")
                nc.scalar.activation(gs[:], gl[:], ACT.Sigmoid,
                                     bias=gbr[:, 0:1])
                po = pop.tile([128, DIM], F32, tag="po")
                for h in range(2):
                    cols = slice(h * 1024, (h + 1) * 1024)
                    nc.vector.tensor_scalar_mul(po[:, cols],
                                                halves[h][:, 0:1024],
                                                gs[:, 0:1])
                    nc.sync.dma_start(pre[rows, cols], po[:, cols])
                sa = scra.tile([128, DIM], F32, tag="sa")
                nc.scalar.activation(sa[:, 0:1024], halves[1][:, 0:1024],
                                     ACT.Abs,
                                     accum_out=acc_av[:, i + 1:i + 2])

            nc.gpsimd.dma_start(accav[:, :], acc_av[:])

    nc.compile()
    return nc


def _get_program():
    if "nc" not in _CACHE:
        _CACHE["nc"] = _build_program()
    return _CACHE["nc"]


def _regulator_host(stress, excitation, fatigue, r1_w, r1_b, ln_g, ln_b,
                    r2_w, r2_b):
    sig = np.array([stress, excitation, fatigue], np.float64)
    h = sig @ np.asarray(r1_w, np.float64).T + np.asarray(r1_b, np.float64)
    mu = h.mean()
    var = h.var()
    h = (h - mu) / np.sqrt(var + LN_EPS) * np.asarray(ln_g, np.float64) \
        + np.asarray(ln_b, np.float64)
    h = np.tanh(h)
    z = h @ np.asarray(r2_w, np.float64).T + np.asarray(r2_b, np.float64)
    return 1.0 / (1.0 + np.exp(-z))


def _host_reference(x, V_w, W_slow_w, gate_w, gate_b, r1_w, r1_b, ln_g,
                    ln_b, r2_w, r2_b, W_fast):
    """Numpy fallback for the (never-hit) W_fast != 0 case."""
    x = x.astype(np.float32)
    v = x @ V_w.T
    stress = x.var(dtype=np.float64).astype(np.float32)
    excitation = np.abs(v).mean(dtype=np.float64).astype(np.float32)
    fatigue = np.float32(np.linalg.norm(W_slow_w))
    ctrl = _regulator_host(stress, excitation, fatigue, r1_w, r1_b, ln_g,
                           ln_b, r2_w, r2_b)
    gate = 1.0 / (1.0 + np.exp(-(v @ gate_w.T + gate_b))) * ctrl[0]
    n = np.float32(x.shape[0])
    y = x @ W_fast.T
    hebb = (y.T @ x) / n
    forget = np.mean(y * y, axis=0)[:, None] * W_fast
    Wf_new = W_fast + np.tanh(hebb - forget) * (ctrl[1] * np.float32(0.1))
    fast_out = x @ Wf_new.T
    return (gate * (v + fast_out * ctrl[2])).astype(np.float32)


def kernel(x, V_w, W_slow_w, gate_w, gate_b, r1_w, r1_b, ln_g, ln_b,
           r2_w, r2_b, W_fast):
    x = np.asarray(x, np.float32)
    V_w = np.asarray(V_w, np.float32)
    W_slow_w = np.asarray(W_slow_w, np.float32)
    gate_w = np.asarray(gate_w, np.float32)
    gate_b = np.asarray(gate_b, np.float32)
    W_fast = np.asarray(W_fast, np.float32)

    if np.any(W_fast):
        return _host_reference(x, V_w, W_slow_w, gate_w, gate_b,
                               np.asarray(r1_w, np.float32),
                               np.asarray(r1_b, np.float32),
                               np.asarray(ln_g, np.float32),
                               np.asarray(ln_b, np.float32),
                               np.asarray(r2_w, np.float32),
                               np.asarray(r2_b, np.float32), W_fast)

    in_maps = _prepare_inmaps(x, V_w, W_slow_w, gate_w, gate_b, r1_w, r1_b,
                              ln_g, ln_b, r2_w, r2_b)
    res = _run(in_maps)

    sumabs = float(sum(res.results[c]["accav"].astype(np.float64).sum()
                       for c in range(NCORES)))
    excitation = sumabs / (float(N) * float(DIM))
    stress = float(x.var(dtype=np.float64))
    fatigue = float(np.linalg.norm(W_slow_w.astype(np.float64)))
    ctrl = _regulator_host(stress, excitation, fatigue, r1_w, r1_b, ln_g,
                           ln_b, r2_w, r2_b)
    strength = np.float32(ctrl[0])

    out = np.concatenate([res.results[c]["pre"] for c in range(NCORES)],
                         axis=0)
    out = (out * strength).astype(np.float32, copy=False)
    return out


def _run(in_maps, **kw):
    from concourse import bass_utils
    nc = _get_program()
    return bass_utils.run_bass_kernel_spmd(nc, in_maps,
                                           core_ids=list(range(NCORES)), **kw)


def _prepare_inmaps(x, V_w, W_slow_w, gate_w, gate_b, r1_w, r1_b, ln_g,
                    ln_b, r2_w, r2_b):
    import ml_dtypes
    bf16 = ml_dtypes.bfloat16

    x = np.asarray(x, np.float32)
    vwt_h = np.ascontiguousarray(np.asarray(V_w, np.float32).T.astype(bf16))
    gw1_h = np.ascontiguousarray(
        np.asarray(gate_w, np.float32).reshape(1, DIM))
    gbrep_h = np.full((128, 1),
                      np.float32(np.asarray(gate_b).reshape(-1)[0]),
                      np.float32)

    in_maps = []
    for c in range(NCORES):
        xs = x[c * RPC:(c + 1) * RPC, :].reshape(ITILES, 128, KTILES, 128)
        # xtl[i*128 + p, t*128 + r] = x_shard[i*128 + r, t*128 + p]
        xtl = np.ascontiguousarray(
            xs.transpose(0, 3, 2, 1).reshape(RPC, DIM).astype(bf16))
        in_maps.append({
            "xtl": xtl,
            "vwt": vwt_h,
            "gw1": gw1_h,
            "gbrep": gbrep_h,
        })
    return in_maps
